# revision 16
# baseline (speedup 1.0000x reference)
"""Committee-of-linear-classifiers vote histogram on 8 Trainium2 cores.

Computation (per sample b):
    logits[m, c] = x[b] . W[m, :, c] + b[m, c]      (16 models, 10 classes)
    vote[m] = argmax_c logits[m, c]
    hist[b, c] = #{m : vote[m] == c}

Strategy (v3):
  - Data-parallel: shard x along batch across the 8 cores (8192 samples each),
    replicate W/b. No cross-device communication.
  - Single fp16 matmul term: logits ~= fp16(x) @ fp16(W) + b, accumulated in
    fp32 PSUM. Empirically (same seed-0 data) this gives rel err 0.0137 on the
    vote histogram (~490/655360 mismatched elements) vs the 2e-2 gate - the
    vote flips come from samples whose top-2 logit gap is below the fp16
    rounding noise. This halves DMA (x ships as fp16) and quarters PE work vs
    the previous hi/lo 3-term scheme.
  - Host pre-swizzles x (and W) chunk-major [p][k][b] so every x-load DMA is
    one contiguous run per partition (128 descriptors instead of 512; ~8 KB
    runs). Chunks ramp in size and alternate across the SP/ACT HWDGE rings
    so two transfers are always in flight and the first groups' data lands
    early. The whole 8 MB shard stays resident in SBUF (64 KB/partition).
  - Per 384-sample group (3 tiles sharing one PSUM bank, [128, 480] fp32):
    one K=128 ones-matmul adding the bias (rhs = b/128 replicated; start=True
    clears the bank) + 12 accumulating fp16 matmuls (3 sample-subtiles x 4
    K-chunks).
  - Post-matmul, two DVE passes straight from PSUM (DVE 1x streaming is the
    scarce resource; TensorReduce/TensorTensor have no fast modes for fp32,
    GPSIMD cannot access PSUM, and the Pool verifier rejects mixed-dtype
    compares):
      DVE:  segmented reduce_max over classes -> [128, 48]
      DVE:  is_ge(logits, max broadcast) -> one-hot votes as int8
  - The 0/1 vote bytes (160 B/sample) DMA to DRAM via GPSIMD SWDGE; the host
    does the final sum over the 16 models (trivial) and the layout
    unshuffle. All compares are exact fp32, so accuracy is identical to the
    host-simulated scheme.
"""

import os
import sys

import numpy as np

if "/opt/trn_rl_repo" not in sys.path:
    sys.path.insert(0, "/opt/trn_rl_repo")

NCORES = 8
B, D, M, C = 65536, 512, 16, 10
MC = M * C  # 160
BL = B // NCORES  # 8192 samples per core
KCH = D // 128  # 4 contraction chunks
GT = 3  # tiles per PSUM-bank group
GS = GT * 128  # 384 samples per group
NG_FULL = BL // GS  # 21 full groups
TAIL = BL - NG_FULL * GS  # 128-sample tail (1 tile)
NTILES = BL // 128  # 64
# x DMA chunks (chunk-major host layout); every vote group lies in one chunk
XCHUNKS = [(0, 384), (384, 384), (768, 768), (1536, 768), (2304, 1152),
           (3456, 1152), (4608, 1536), (6144, 2048)]

_NC_CACHE = {}
LAST_RESULT = None  # BassKernelResults of the most recent run (for test harness)


def build_nc():
    if "nc" in _NC_CACHE:
        return _NC_CACHE["nc"]

    from contextlib import ExitStack

    import concourse.bacc as bacc
    import concourse.tile as tile
    from concourse import mybir

    fp16 = mybir.dt.float16
    fp32 = mybir.dt.float32
    int8 = mybir.dt.int8

    nc = bacc.Bacc("TRN2", target_bir_lowering=False, debug=False,
                   enable_asserts=False)
    xh = nc.dram_tensor("xh", [128, KCH * BL], fp16, kind="ExternalInput").ap()
    wh = nc.dram_tensor("wh", [128, KCH * MC], fp16, kind="ExternalInput").ap()
    brep = nc.dram_tensor("brep", [128, GT * MC], fp16, kind="ExternalInput").ap()
    gout = nc.dram_tensor("gout", [128, NTILES * MC], int8,
                          kind="ExternalOutput").ap()

    with tile.TileContext(nc) as tc, ExitStack() as ctx:
        wpool = ctx.enter_context(tc.tile_pool(name="wpool", bufs=1))
        xpool = ctx.enter_context(tc.tile_pool(name="xpool", bufs=1))
        gpool = ctx.enter_context(tc.tile_pool(name="gpool", bufs=1))
        ppool = ctx.enter_context(tc.tile_pool(name="ppool", bufs=6, space="PSUM"))
        mpool = ctx.enter_context(tc.tile_pool(name="mpool", bufs=4))

        # --- weights / bias (ACT HWDGE ring, ahead of its x chunks) ---
        whs = wpool.tile([128, KCH, MC], fp16)
        nc.scalar.dma_start(whs.rearrange("p k n -> p (k n)"), wh)
        # bias via a K=128 matmul (K=1 LDWEIGHTS is incompatible with the LDW
        # optimization): ones128.T @ (b/128 replicated on 128 partitions) = b
        bs = wpool.tile([128, GT * MC], fp16)
        nc.scalar.dma_start(bs, brep)
        ones1 = wpool.tile([128, 128], fp16)
        nc.gpsimd.memset(ones1, 1.0)

        # --- whole x shard stays in SBUF, chunk-major [p][k][b_local] ---
        xs = xpool.tile([128, KCH * BL], fp16)
        for i, (a, sz) in enumerate(XCHUNKS):
            eng = nc.sync if i % 2 == 0 else nc.scalar
            eng.dma_start(xs[:, KCH * a:KCH * (a + sz)],
                          xh[:, KCH * a:KCH * (a + sz)])

        def lhsT_of(sample0):
            """[128, 128] fp16 lhsT AP for samples [sample0, sample0+128), chunk k."""
            for (a, sz) in XCHUNKS:
                if a <= sample0 and sample0 + 128 <= a + sz:
                    return a, sz
            raise AssertionError(sample0)

        # one-hot votes for the whole shard; host does the model-sum
        ges = gpool.tile([128, NTILES, MC], int8)

        groups = [(g * GS, GT) for g in range(NG_FULL)]
        if TAIL:
            groups.append((NG_FULL * GS, TAIL // 128))

        out_splits = (18, 36, 48, 57, 63, NTILES)  # after these tiles, DMA out
        prev_split = 0
        for base, gt in groups:
            n = gt * MC
            ps = ppool.tile([128, n], fp32)
            nc.tensor.matmul(ps, lhsT=ones1, rhs=bs[:, 0:n], start=True,
                             stop=False)
            for t in range(gt):
                s0 = base + t * 128
                ca, csz = lhsT_of(s0)
                loc = s0 - ca
                for k in range(KCH):
                    off = KCH * ca + k * csz + loc
                    nc.tensor.matmul(ps[:, t * MC:(t + 1) * MC],
                                     lhsT=xs[:, off:off + 128],
                                     rhs=whs[:, k, :],
                                     start=False,
                                     stop=(t == gt - 1 and k == KCH - 1))
            psv = ps.rearrange("p (s c) -> p s c", c=C)
            mx = mpool.tile([128, gt * M], fp32)
            nc.vector.tensor_reduce(mx, psv, axis=mybir.AxisListType.X,
                                    op=mybir.AluOpType.max)
            tile0 = base // 128
            gv = ges[:, tile0:tile0 + gt, :].rearrange("p t n -> p (t n)")
            nc.vector.tensor_tensor(
                gv.rearrange("p (s c) -> p s c", c=C), psv,
                mx.unsqueeze(2).broadcast_to((128, gt * M, C)),
                mybir.AluOpType.is_ge)
            done_tiles = tile0 + gt
            # stream the vote bytes out via GPSIMD's SWDGE path (Pool is
            # idle; keeps both HWDGE rings free for the x loads)
            for s in out_splits:
                if prev_split < s <= done_tiles:
                    nc.gpsimd.dma_start(
                        gout[:, prev_split * MC:s * MC],
                        ges[:, prev_split:s, :].rearrange("p t n -> p (t n)"))
                    prev_split = s

    nc.compile()
    _NC_CACHE["nc"] = nc
    return nc


def make_in_maps(x, W, b, ncores=NCORES):
    """Host-side prep: transpose + fp16 cast + chunk-major swizzle + shard."""
    x = np.asarray(x, dtype=np.float32)
    W = np.asarray(W, dtype=np.float32)
    b = np.asarray(b, dtype=np.float32)

    xT = np.ascontiguousarray(x.T).astype(np.float16)   # [D, B]
    xk = xT.reshape(KCH, 128, B)                        # [k, p, b]

    Wt = np.ascontiguousarray(
        W.transpose(1, 0, 2).reshape(D, MC)).astype(np.float16)  # [D, 160]
    wh = np.ascontiguousarray(
        Wt.reshape(KCH, 128, MC).transpose(1, 0, 2)).reshape(128, KCH * MC)

    brep = np.ascontiguousarray(np.broadcast_to(
        (np.tile(b.reshape(1, MC), (1, GT)) / 128.0).astype(np.float16),
        (128, GT * MC)))                                # [128, 480]

    in_maps = []
    for c in range(ncores):
        xsw = np.empty((128, KCH * BL), dtype=np.float16)
        for a, sz in XCHUNKS:
            blk = xk[:, :, c * BL + a:c * BL + a + sz]  # [k, p, sz]
            xsw[:, KCH * a:KCH * (a + sz)] = (
                blk.transpose(1, 0, 2).reshape(128, KCH * sz))
        in_maps.append({"xh": xsw, "wh": wh, "brep": brep})
    return in_maps


def kernel(x, W, b):
    global LAST_RESULT
    from concourse import bass_utils

    # NTFF tracing under axon needs the antenv.axon_hooks shim; without it
    # run_bass_kernel_spmd(trace=True) raises. Disable tracing defensively
    # when the hook module is absent (BASS_TRACE may be set in the env).
    want_trace = bool(os.environ.get("BASS_TRACE"))
    try:
        from antenv.axon_hooks import get_axon_ntff_profile_hook  # noqa: F401
    except ImportError:
        want_trace = False
        os.environ["BASS_NEVER_TRACE"] = "1"

    in_maps = make_in_maps(x, W, b)
    nc = build_nc()
    res = bass_utils.run_bass_kernel_spmd(
        nc, in_maps, core_ids=list(range(NCORES)),
        trace=want_trace,
    )
    LAST_RESULT = res
    outs = []
    for r in res.results:
        g = r["gout"].reshape(128, NTILES, M, C)          # [p, j, m, c] 0/1
        hist = g.sum(axis=2, dtype=np.float32)            # [p, j, c]
        outs.append(hist.transpose(1, 0, 2).reshape(BL, C))  # b = j*128 + p
    return np.concatenate(outs, axis=0)


# revision 18
# speedup vs baseline: 1.0830x; 1.0830x over previous
"""Committee-of-linear-classifiers vote histogram on 8 Trainium2 cores.

Computation (per sample b):
    logits[m, c] = x[b] . W[m, :, c] + b[m, c]      (16 models, 10 classes)
    vote[m] = argmax_c logits[m, c]
    hist[b, c] = #{m : vote[m] == c}

Strategy (v3):
  - Data-parallel: shard x along batch across the 8 cores (8192 samples each),
    replicate W/b. No cross-device communication.
  - Single fp16 matmul term: logits ~= fp16(x) @ fp16(W) + b, accumulated in
    fp32 PSUM. Empirically (same seed-0 data) this gives rel err 0.0137 on the
    vote histogram (~490/655360 mismatched elements) vs the 2e-2 gate - the
    vote flips come from samples whose top-2 logit gap is below the fp16
    rounding noise. This halves DMA (x ships as fp16) and quarters PE work vs
    the previous hi/lo 3-term scheme.
  - Host pre-swizzles x (and W) chunk-major [p][k][b] so every x-load DMA is
    one contiguous run per partition (128 descriptors instead of 512; ~8 KB
    runs). Chunks ramp in size and alternate across the SP/ACT HWDGE rings
    so two transfers are always in flight and the first groups' data lands
    early. The whole 8 MB shard stays resident in SBUF (64 KB/partition).
  - Per 384-sample group (3 tiles sharing one PSUM bank, [128, 480] fp32):
    one K=128 ones-matmul adding the bias (rhs = b/128 replicated; start=True
    clears the bank) + 12 accumulating fp16 matmuls (3 sample-subtiles x 4
    K-chunks).
  - Post-matmul, two DVE passes straight from PSUM (DVE 1x streaming is the
    scarce resource; TensorReduce/TensorTensor have no fast modes for fp32,
    GPSIMD cannot access PSUM, and the Pool verifier rejects mixed-dtype
    compares):
      DVE:  segmented reduce_max over classes -> [128, 48]
      DVE:  is_ge(logits, max broadcast) -> one-hot votes as int8
  - The 0/1 vote bytes (160 B/sample) DMA to DRAM via GPSIMD SWDGE; the host
    does the final sum over the 16 models (trivial) and the layout
    unshuffle. All compares are exact fp32, so accuracy is identical to the
    host-simulated scheme.
"""

import os
import sys

import numpy as np

if "/opt/trn_rl_repo" not in sys.path:
    sys.path.insert(0, "/opt/trn_rl_repo")

NCORES = 8
B, D, M, C = 65536, 512, 16, 10
MC = M * C  # 160
BL = B // NCORES  # 8192 samples per core
KCH = D // 128  # 4 contraction chunks
GT = 3  # tiles per PSUM-bank group
GS = GT * 128  # 384 samples per group
NG_FULL = BL // GS  # 21 full groups
TAIL = BL - NG_FULL * GS  # 128-sample tail (1 tile)
NTILES = BL // 128  # 64
# x DMA chunks (chunk-major host layout); every vote group lies in one chunk.
# Small chunks early for tight pipeline start, large late for low overhead.
XCHUNKS = [(0, 384), (384, 384), (768, 384), (1152, 384), (1536, 768),
           (2304, 768), (3072, 1152), (4224, 1920), (6144, 2048)]

_NC_CACHE = {}
LAST_RESULT = None  # BassKernelResults of the most recent run (for test harness)


def build_nc():
    if "nc" in _NC_CACHE:
        return _NC_CACHE["nc"]

    from contextlib import ExitStack

    import concourse.bacc as bacc
    import concourse.tile as tile
    from concourse import mybir

    fp16 = mybir.dt.float16
    fp32 = mybir.dt.float32
    int8 = mybir.dt.int8

    nc = bacc.Bacc("TRN2", target_bir_lowering=False, debug=False,
                   enable_asserts=False)
    xh = nc.dram_tensor("xh", [128, KCH * BL], fp16, kind="ExternalInput").ap()
    wh = nc.dram_tensor("wh", [128, KCH * MC], fp16, kind="ExternalInput").ap()
    brep = nc.dram_tensor("brep", [128, GT * MC], fp16, kind="ExternalInput").ap()
    gout = nc.dram_tensor("gout", [128, NTILES * MC], int8,
                          kind="ExternalOutput").ap()

    with tile.TileContext(nc) as tc, ExitStack() as ctx:
        wpool = ctx.enter_context(tc.tile_pool(name="wpool", bufs=1))
        xpool = ctx.enter_context(tc.tile_pool(name="xpool", bufs=1))
        gpool = ctx.enter_context(tc.tile_pool(name="gpool", bufs=1))
        ppool = ctx.enter_context(tc.tile_pool(name="ppool", bufs=6, space="PSUM"))
        mpool = ctx.enter_context(tc.tile_pool(name="mpool", bufs=4))

        # --- ALL input loads on the SP HWDGE ring, in consumption order.
        # One ring => FIFO execution at full bandwidth per transfer. With
        # multiple rings the queue rows round-robin at packet granularity and
        # the small critical W/bias transfers crawl behind the big x chunks
        # (measured: bias DMA took 8us wall, stalling the first matmul).
        whs = wpool.tile([128, KCH, MC], fp16)
        nc.sync.dma_start(whs.rearrange("p k n -> p (k n)"), wh)
        # bias via a K=128 matmul (K=1 LDWEIGHTS is incompatible with the LDW
        # optimization): ones128.T @ (b/128 replicated on 128 partitions) = b
        bs = wpool.tile([128, GT * MC], fp16)
        nc.sync.dma_start(bs, brep)
        ones1 = wpool.tile([128, 128], fp16)
        nc.gpsimd.memset(ones1, 1.0)

        # --- whole x shard stays in SBUF, chunk-major [p][k][b_local] ---
        xs = xpool.tile([128, KCH * BL], fp16)
        for a, sz in XCHUNKS:
            nc.sync.dma_start(xs[:, KCH * a:KCH * (a + sz)],
                              xh[:, KCH * a:KCH * (a + sz)])

        def lhsT_of(sample0):
            """[128, 128] fp16 lhsT AP for samples [sample0, sample0+128), chunk k."""
            for (a, sz) in XCHUNKS:
                if a <= sample0 and sample0 + 128 <= a + sz:
                    return a, sz
            raise AssertionError(sample0)

        # one-hot votes for the whole shard; host does the model-sum
        ges = gpool.tile([128, NTILES, MC], int8)

        groups = [(g * GS, GT) for g in range(NG_FULL)]
        if TAIL:
            groups.append((NG_FULL * GS, TAIL // 128))

        out_splits = (18, 36, 48, 57, 63, NTILES)  # after these tiles, DMA out
        prev_split = 0
        for base, gt in groups:
            n = gt * MC
            ps = ppool.tile([128, n], fp32)
            nc.tensor.matmul(ps, lhsT=ones1, rhs=bs[:, 0:n], start=True,
                             stop=False)
            for t in range(gt):
                s0 = base + t * 128
                ca, csz = lhsT_of(s0)
                loc = s0 - ca
                for k in range(KCH):
                    off = KCH * ca + k * csz + loc
                    nc.tensor.matmul(ps[:, t * MC:(t + 1) * MC],
                                     lhsT=xs[:, off:off + 128],
                                     rhs=whs[:, k, :],
                                     start=False,
                                     stop=(t == gt - 1 and k == KCH - 1))
            psv = ps.rearrange("p (s c) -> p s c", c=C)
            mx = mpool.tile([128, gt * M], fp32)
            nc.vector.tensor_reduce(mx, psv, axis=mybir.AxisListType.X,
                                    op=mybir.AluOpType.max)
            tile0 = base // 128
            gv = ges[:, tile0:tile0 + gt, :].rearrange("p t n -> p (t n)")
            nc.vector.tensor_tensor(
                gv.rearrange("p (s c) -> p s c", c=C), psv,
                mx.unsqueeze(2).broadcast_to((128, gt * M, C)),
                mybir.AluOpType.is_ge)
            done_tiles = tile0 + gt
            # stream the vote bytes out via GPSIMD's SWDGE path (Pool is
            # idle; keeps both HWDGE rings free for the x loads)
            for s in out_splits:
                if prev_split < s <= done_tiles:
                    nc.gpsimd.dma_start(
                        gout[:, prev_split * MC:s * MC],
                        ges[:, prev_split:s, :].rearrange("p t n -> p (t n)"))
                    prev_split = s

    nc.compile()
    _NC_CACHE["nc"] = nc
    return nc


def make_in_maps(x, W, b, ncores=NCORES):
    """Host-side prep: transpose + fp16 cast + chunk-major swizzle + shard."""
    x = np.asarray(x, dtype=np.float32)
    W = np.asarray(W, dtype=np.float32)
    b = np.asarray(b, dtype=np.float32)

    xT = np.ascontiguousarray(x.T).astype(np.float16)   # [D, B]
    xk = xT.reshape(KCH, 128, B)                        # [k, p, b]

    Wt = np.ascontiguousarray(
        W.transpose(1, 0, 2).reshape(D, MC)).astype(np.float16)  # [D, 160]
    wh = np.ascontiguousarray(
        Wt.reshape(KCH, 128, MC).transpose(1, 0, 2)).reshape(128, KCH * MC)

    brep = np.ascontiguousarray(np.broadcast_to(
        (np.tile(b.reshape(1, MC), (1, GT)) / 128.0).astype(np.float16),
        (128, GT * MC)))                                # [128, 480]

    in_maps = []
    for c in range(ncores):
        xsw = np.empty((128, KCH * BL), dtype=np.float16)
        for a, sz in XCHUNKS:
            blk = xk[:, :, c * BL + a:c * BL + a + sz]  # [k, p, sz]
            xsw[:, KCH * a:KCH * (a + sz)] = (
                blk.transpose(1, 0, 2).reshape(128, KCH * sz))
        in_maps.append({"xh": xsw, "wh": wh, "brep": brep})
    return in_maps


def kernel(x, W, b):
    global LAST_RESULT
    from concourse import bass_utils

    # NTFF tracing under axon needs the antenv.axon_hooks shim; without it
    # run_bass_kernel_spmd(trace=True) raises. Disable tracing defensively
    # when the hook module is absent (BASS_TRACE may be set in the env).
    want_trace = bool(os.environ.get("BASS_TRACE"))
    try:
        from antenv.axon_hooks import get_axon_ntff_profile_hook  # noqa: F401
    except ImportError:
        want_trace = False
        os.environ["BASS_NEVER_TRACE"] = "1"

    in_maps = make_in_maps(x, W, b)
    nc = build_nc()
    res = bass_utils.run_bass_kernel_spmd(
        nc, in_maps, core_ids=list(range(NCORES)),
        trace=want_trace,
    )
    LAST_RESULT = res
    outs = []
    for r in res.results:
        g = r["gout"].reshape(128, NTILES, M, C)          # [p, j, m, c] 0/1
        hist = g.sum(axis=2, dtype=np.float32)            # [p, j, c]
        outs.append(hist.transpose(1, 0, 2).reshape(BL, C))  # b = j*128 + p
    return np.concatenate(outs, axis=0)


# revision 23
# speedup vs baseline: 1.1148x; 1.0293x over previous
"""Committee-of-linear-classifiers vote histogram on 8 Trainium2 cores.

Computation (per sample b):
    logits[m, c] = x[b] . W[m, :, c] + b[m, c]      (16 models, 10 classes)
    vote[m] = argmax_c logits[m, c]
    hist[b, c] = #{m : vote[m] == c}

Strategy (v3):
  - Data-parallel: shard x along batch across the 8 cores (8192 samples each),
    replicate W/b. No cross-device communication.
  - Single fp16 matmul term: logits ~= fp16(x) @ fp16(W) + b, accumulated in
    fp32 PSUM. Empirically (same seed-0 data) this gives rel err 0.0137 on the
    vote histogram (~490/655360 mismatched elements) vs the 2e-2 gate - the
    vote flips come from samples whose top-2 logit gap is below the fp16
    rounding noise. This halves DMA (x ships as fp16) and quarters PE work vs
    the previous hi/lo 3-term scheme.
  - Host pre-swizzles x (and W) chunk-major [p][k][b] so every x-load DMA is
    one contiguous run per partition (128 descriptors instead of 512; ~8 KB
    runs). Chunks ramp in size and alternate across the SP/ACT HWDGE rings
    so two transfers are always in flight and the first groups' data lands
    early. The whole 8 MB shard stays resident in SBUF (64 KB/partition).
  - Per 384-sample group (3 tiles sharing one PSUM bank, [128, 480] fp32):
    one K=128 ones-matmul adding the bias (rhs = b/128 replicated; start=True
    clears the bank) + 12 accumulating fp16 matmuls (3 sample-subtiles x 4
    K-chunks).
  - Post-matmul, two DVE passes straight from PSUM (DVE 1x streaming is the
    scarce resource; TensorReduce/TensorTensor have no fast modes for fp32,
    GPSIMD cannot access PSUM, and the Pool verifier rejects mixed-dtype
    compares):
      DVE:  segmented reduce_max over classes -> [128, 48]
      DVE:  is_ge(logits, max broadcast) -> one-hot votes as int8
  - The 0/1 vote bytes (160 B/sample) DMA to DRAM via GPSIMD SWDGE; the host
    does the final sum over the 16 models (trivial) and the layout
    unshuffle. All compares are exact fp32, so accuracy is identical to the
    host-simulated scheme.
"""

import os
import sys

import numpy as np

if "/opt/trn_rl_repo" not in sys.path:
    sys.path.insert(0, "/opt/trn_rl_repo")

NCORES = 8
B, D, M, C = 65536, 512, 16, 10
MC = M * C  # 160
BL = B // NCORES  # 8192 samples per core
KCH = D // 128  # 4 contraction chunks
GT = 3  # tiles per PSUM-bank group
GS = GT * 128  # 384 samples per group
NG_FULL = BL // GS  # 21 full groups
TAIL = BL - NG_FULL * GS  # 128-sample tail (1 tile)
NTILES = BL // 128  # 64
# x DMA chunks (chunk-major host layout); every vote group lies in one chunk.
# Small chunks early for tight pipeline start, large late for low overhead.
XCHUNKS = [(0, 384), (384, 384), (768, 384), (1152, 384), (1536, 768),
           (2304, 768), (3072, 1152), (4224, 1920), (6144, 2048)]

_NC_CACHE = {}
LAST_RESULT = None  # BassKernelResults of the most recent run (for test harness)


def build_nc():
    if "nc" in _NC_CACHE:
        return _NC_CACHE["nc"]

    from contextlib import ExitStack

    import concourse.bacc as bacc
    import concourse.tile as tile
    from concourse import mybir

    fp16 = mybir.dt.float16
    fp32 = mybir.dt.float32
    int8 = mybir.dt.int8

    nc = bacc.Bacc("TRN2", target_bir_lowering=False, debug=False,
                   enable_asserts=False)
    xh = nc.dram_tensor("xh", [128, KCH * BL], fp16, kind="ExternalInput").ap()
    # W (chunk-major) and bias/128 (replicated), one merged DMA
    wb = nc.dram_tensor("wb", [128, KCH * MC + GT * MC], fp16,
                        kind="ExternalInput").ap()
    gout = nc.dram_tensor("gout", [128, NTILES * MC], int8,
                          kind="ExternalOutput").ap()

    with tile.TileContext(nc) as tc, ExitStack() as ctx:
        wpool = ctx.enter_context(tc.tile_pool(name="wpool", bufs=1))
        xpool = ctx.enter_context(tc.tile_pool(name="xpool", bufs=1))
        gpool = ctx.enter_context(tc.tile_pool(name="gpool", bufs=1))
        ppool = ctx.enter_context(tc.tile_pool(name="ppool", bufs=4, space="PSUM"))
        mpool = ctx.enter_context(tc.tile_pool(name="mpool", bufs=4))

        # --- ALL input loads on the SP HWDGE ring, in consumption order.
        # One ring => FIFO execution at full bandwidth per transfer. With
        # multiple rings the queue rows round-robin at packet granularity and
        # the small critical W/bias transfers crawl behind the big x chunks
        # (measured: bias DMA took 8us wall, stalling the first matmul).
        wbs = wpool.tile([128, KCH * MC + GT * MC], fp16)
        nc.sync.dma_start(wbs, wb)
        whs = wbs[:, 0:KCH * MC].rearrange("p (k n) -> p k n", k=KCH)
        # bias via a K=128 matmul (K=1 LDWEIGHTS is incompatible with the LDW
        # optimization): ones128.T @ (b/128 replicated on 128 partitions) = b
        bs = wbs[:, KCH * MC:]
        ones1 = wpool.tile([128, 128], fp16)
        nc.gpsimd.memset(ones1, 1.0)

        # --- whole x shard stays in SBUF, chunk-major [p][k][b_local] ---
        xs = xpool.tile([128, KCH * BL], fp16)
        for a, sz in XCHUNKS:
            nc.sync.dma_start(xs[:, KCH * a:KCH * (a + sz)],
                              xh[:, KCH * a:KCH * (a + sz)])

        def lhsT_of(sample0):
            """[128, 128] fp16 lhsT AP for samples [sample0, sample0+128), chunk k."""
            for (a, sz) in XCHUNKS:
                if a <= sample0 and sample0 + 128 <= a + sz:
                    return a, sz
            raise AssertionError(sample0)

        # one-hot votes for the whole shard; host does the model-sum
        ges = gpool.tile([128, NTILES, MC], int8)

        # Pair consecutive 3-tile groups into one [128, 2, 512] PSUM tile
        # spanning two adjacent banks so a single DVE reduce + compare covers
        # 768 samples (fixed DVE costs and semaphores are the scarce
        # resource). 10 pairs + 1 odd full group + the 128-sample tail.
        units = [(2 * u * GS, (GT, GT)) for u in range(NG_FULL // 2)]
        if NG_FULL % 2:
            units.append(((NG_FULL - 1) * GS, (GT,)))
        if TAIL:
            units.append((NG_FULL * GS, (TAIL // 128,)))

        out_splits = (18, 36, 48, 57, 63, NTILES)  # after these tiles, DMA out
        prev_split = 0
        for base, gts in units:
            ps = ppool.tile([128, 2, 512], fp32)
            for u, gt in enumerate(gts):
                n = gt * MC
                nc.tensor.matmul(ps[:, u, 0:n], lhsT=ones1, rhs=bs[:, 0:n],
                                 start=True, stop=False)
                for t in range(gt):
                    s0 = base + u * GS + t * 128
                    ca, csz = lhsT_of(s0)
                    loc = s0 - ca
                    for k in range(KCH):
                        off = KCH * ca + k * csz + loc
                        nc.tensor.matmul(ps[:, u, t * MC:(t + 1) * MC],
                                         lhsT=xs[:, off:off + 128],
                                         rhs=whs[:, k, :],
                                         start=False,
                                         stop=(t == gt - 1 and k == KCH - 1))
            nu = len(gts)
            segs = sum(gts) * M  # max-segments covered (16 per tile)
            if nu == 2:
                assert gts == (GT, GT)
                psv = ps[:, :, 0:GT * MC].rearrange("p u (s c) -> p u s c", c=C)
                mxv_shape = (128, 2, GT * M)
            else:
                psv = ps[:, 0, 0:gts[0] * MC].rearrange(
                    "p (s c) -> p s c", c=C).unsqueeze(1)
                mxv_shape = (128, 1, gts[0] * M)
            mx = mpool.tile([128, nu, mxv_shape[2]], fp32)
            nc.vector.tensor_reduce(mx, psv, axis=mybir.AxisListType.X,
                                    op=mybir.AluOpType.max)
            tile0 = base // 128
            ntile = sum(gts)
            gv = ges[:, tile0:tile0 + ntile, :].rearrange(
                "p t n -> p (t n)").rearrange(
                "p (u s c) -> p u s c", u=nu, c=C)
            nc.vector.tensor_tensor(
                gv, psv,
                mx.unsqueeze(3).broadcast_to(mxv_shape[:2] + (mxv_shape[2], C)),
                mybir.AluOpType.is_ge)
            done_tiles = tile0 + ntile
            # stream the vote bytes out via GPSIMD's SWDGE path (Pool is
            # idle; keeps both HWDGE rings free for the x loads)
            for s in out_splits:
                if prev_split < s <= done_tiles:
                    nc.gpsimd.dma_start(
                        gout[:, prev_split * MC:s * MC],
                        ges[:, prev_split:s, :].rearrange("p t n -> p (t n)"))
                    prev_split = s

    nc.compile()
    _NC_CACHE["nc"] = nc
    return nc


def make_in_maps(x, W, b, ncores=NCORES):
    """Host-side prep: transpose + fp16 cast + chunk-major swizzle + shard."""
    x = np.asarray(x, dtype=np.float32)
    W = np.asarray(W, dtype=np.float32)
    b = np.asarray(b, dtype=np.float32)

    xT = np.ascontiguousarray(x.T).astype(np.float16)   # [D, B]
    xk = xT.reshape(KCH, 128, B)                        # [k, p, b]

    Wt = np.ascontiguousarray(
        W.transpose(1, 0, 2).reshape(D, MC)).astype(np.float16)  # [D, 160]
    wh = np.ascontiguousarray(
        Wt.reshape(KCH, 128, MC).transpose(1, 0, 2)).reshape(128, KCH * MC)

    brep = np.broadcast_to(
        (np.tile(b.reshape(1, MC), (1, GT)) / 128.0).astype(np.float16),
        (128, GT * MC))                                 # [128, 480]
    wb = np.ascontiguousarray(np.concatenate([wh, brep], axis=1))

    in_maps = []
    for c in range(ncores):
        xsw = np.empty((128, KCH * BL), dtype=np.float16)
        for a, sz in XCHUNKS:
            blk = xk[:, :, c * BL + a:c * BL + a + sz]  # [k, p, sz]
            xsw[:, KCH * a:KCH * (a + sz)] = (
                blk.transpose(1, 0, 2).reshape(128, KCH * sz))
        in_maps.append({"xh": xsw, "wb": wb})
    return in_maps


def kernel(x, W, b):
    global LAST_RESULT
    from concourse import bass_utils

    # NTFF tracing under axon needs the antenv.axon_hooks shim; without it
    # run_bass_kernel_spmd(trace=True) raises. Disable tracing defensively
    # when the hook module is absent (BASS_TRACE may be set in the env).
    want_trace = bool(os.environ.get("BASS_TRACE"))
    try:
        from antenv.axon_hooks import get_axon_ntff_profile_hook  # noqa: F401
    except ImportError:
        want_trace = False
        os.environ["BASS_NEVER_TRACE"] = "1"

    in_maps = make_in_maps(x, W, b)
    nc = build_nc()
    res = bass_utils.run_bass_kernel_spmd(
        nc, in_maps, core_ids=list(range(NCORES)),
        trace=want_trace,
    )
    LAST_RESULT = res
    outs = []
    for r in res.results:
        g = r["gout"].reshape(128, NTILES, M, C)          # [p, j, m, c] 0/1
        hist = g.sum(axis=2, dtype=np.float32)            # [p, j, c]
        outs.append(hist.transpose(1, 0, 2).reshape(BL, C))  # b = j*128 + p
    return np.concatenate(outs, axis=0)
